# revision 14
# baseline (speedup 1.0000x reference)
"""EntropyGraph Trainium2 kernel (v2).

Computes, per batch b (one NeuronCore per batch):
  qt = heads(queries @ Wq_w.T + Wq_b), kt = heads(keys @ Wk_w.T + Wk_b)
  out[b,h,i,j] = -0.5 * sum_m log(1 - corr_m(i,j)^2 + eps)
where corr_m is the lag-m cross-correlation between query series i and key
series j within each head.

Structure vs v1:
  - corr = alpha_i * G[i,j]; G = PE Gram of (raw q rows + mean-aug row)
    against (beta-scaled k rows + -s1y-aug rows). One-sided centering makes
    the mean correction exact.
  - Per iteration t = 8h+ic the two Gram PSUM tiles are evacuated as
      v1 = (a1*G1)^2  (ACT Square, scale=a1 per partition)
      v2 = A2*G2^2    (single DVE scalar_tensor_tensor, A=a^2; a fraction
                       of iterations ride ACT instead to balance engines)
    then u = (v1-C)(v2-C) in 2 DVE f16 ops (pair-batched), one ACT Ln over
    a quad [128,4096], and a -0.5 DVE tensor_scalar (quad, 4x mode).
  - All ACT functions (Identity/Square/Ln/Exp) live in one table set;
    rsqrt is computed as exp(-0.5*ln(x)) so no table switches occur.
  - Inputs are cast to f32r so every matmul runs at 1 cycle/row.
"""

import sys

import numpy as np

sys.path.insert(0, "/opt/trn_rl_repo")

import concourse.bacc as bacc
import concourse.tile as tile
from concourse import mybir
from concourse.bass_utils import run_bass_kernel_spmd

F32 = mybir.dt.float32
F32R = mybir.dt.float32r
F16 = mybir.dt.float16
OP = mybir.AluOpType
AF = mybir.ActivationFunctionType

B, N, DF = 8, 1024, 128
H, DK = 8, 64
EPS = 1e-6
C = 1.0 + EPS
NCHUNK = 4
# evac2 rides ACT (instead of DVE) when t % 8 in this set: engine balance
_ACT_EVAC2_SLOTS = (3, 7)
# on the DVE evac2 path, the f16 squaring op goes to Pool when t % 8 in
# this set (Pool Multiply eff 0.42 but otherwise idle in steady state)
_POOL_SQ_SLOTS = (0, 2, 4)


def _emit_body(nc, tc, t):
    qT, kT, wqT, wkT, bq, bk, xmask, ymask, invn, ident, out, betad = t
    with tc.tile_pool(name="const", bufs=1) as const, \
         tc.tile_pool(name="proj", bufs=1) as projp, \
         tc.tile_pool(name="stats", bufs=1) as statp:

        # statp: tiles that stage E reads; everything else transient.
        ns1y = statp.tile([16, N], F32R)
        mx = statp.tile([16, N], F32R)
        aT = statp.tile([128, 128], F32)

        invn_s = const.tile([16, 1], F32)
        id_s = const.tile([128, 128], F32)

        with tc.tile_pool(name="inp", bufs=1) as inp, \
             tc.tile_pool(name="statd", bufs=1) as statd:
            # ---- Stage A: load inputs ---------------------------------
            qT_s = inp.tile([DF, N], F32)
            kT_s = inp.tile([DF, N], F32)
            wqT_s = inp.tile([DF, 512], F32)
            wkT_s = inp.tile([DF, 512], F32)
            bq_s = inp.tile([128, 4], F32)
            bk_s = inp.tile([128, 4], F32)
            xm_s = inp.tile([128, 64], F32)
            ym_s = inp.tile([128, 64], F32)
            for dst, src in ((kT_s, kT), (wkT_s, wkT), (qT_s, qT),
                             (wqT_s, wqT), (bq_s, bq), (bk_s, bk),
                             (xm_s, xmask), (ym_s, ymask), (invn_s, invn),
                             (id_s, ident)):
                nc.sync.dma_start(out=dst, in_=src[:, :])

            # f32r rounding casts (the verifier rejects raw-DMA data as
            # f32r matmul input). k-side first: beta gates stage E head 0.
            kT_r = inp.tile([DF, N], F32R)
            wkT_r = inp.tile([DF, 512], F32R)
            qT_r = inp.tile([DF, N], F32R)
            wqT_r = inp.tile([DF, 512], F32R)
            xm_r = inp.tile([128, 64], F32R)
            ym_r = inp.tile([128, 64], F32R)
            nc.vector.tensor_copy(kT_r, kT_s)
            nc.scalar.copy(wkT_r, wkT_s)
            nc.vector.tensor_copy(qT_r, qT_s)
            nc.scalar.copy(wqT_r, wqT_s)
            nc.scalar.copy(ym_r, ym_s)
            nc.scalar.copy(xm_r, xm_s)

            # ---- Stage B: projections (transposed layout) -------------
            # projT[o, n] = W[o, :] @ inT[:, n] + b[o]; f32r matmul (via
            # bitcast: 1 cycle/row vs 4 for fp32), evac adds the
            # per-partition bias and rounds to f32r (ACT/DVE alternate).
            # k-side first: beta gates stage E head 0.
            qproj = []
            kproj = []
            sq_list = {}
            with tc.tile_pool(name="sqp", bufs=1) as sqp:
                with tc.tile_pool(name="pps", bufs=2, space="PSUM") as pps:
                    for (src_r, w_r, b_s, dst_list, pname) in (
                            (kT_r, wkT_r, bk_s, kproj, "k"),
                            (qT_r, wqT_r, bq_s, qproj, "q")):
                        for c in range(NCHUNK):
                            psb = pps.tile([128, N], F32, tag="pps")
                            for jh in range(2):
                                nc.tensor.matmul(
                                    psb[:, jh * 512:(jh + 1) * 512],
                                    lhsT=w_r[:, c * 128:(c + 1) * 128],
                                    rhs=src_r[:, jh * 512:(jh + 1) * 512],
                                    start=True, stop=True)
                            pt = projp.tile([128, N], F32R,
                                            tag=f"proj_{pname}_{c}")
                            if c % 2 == 1:
                                nc.vector.tensor_scalar(
                                    out=pt, in0=psb, scalar1=1.0,
                                    scalar2=b_s[:, c:c + 1],
                                    op0=OP.mult, op1=OP.add)
                            else:
                                nc.scalar.activation(
                                    out=pt, in_=psb, func=AF.Identity,
                                    bias=b_s[:, c:c + 1], scale=1.0)
                            dst_list.append(pt)
                            # squared projections for the s2 stats; from
                            # SBUF so Pool can carry half of them
                            sq = sqp.tile([128, N], F32R,
                                          tag=f"sq{pname}{c}")
                            if c % 2 == 0:
                                nc.gpsimd.tensor_mul(sq, pt, pt)
                            else:
                                nc.vector.tensor_mul(sq, pt, pt)
                            sq_list[(pname, c)] = sq

                # ---- Stage C: raw moments via mask matmuls ------------
                stats_sb = {}
                with tc.tile_pool(name="sps", bufs=1, space="PSUM") as sps:
                    for name, plist, mask in (("k", kproj, ym_r),
                                              ("q", qproj, xm_r)):
                        ps1 = sps.tile([16, N], F32, tag=f"ps1{name}")
                        ps2 = sps.tile([16, N], F32, tag=f"ps2{name}")
                        for c in range(NCHUNK):
                            for jh in range(2):
                                sl = slice(jh * 512, (jh + 1) * 512)
                                nc.tensor.matmul(
                                    ps1[:, sl],
                                    lhsT=mask[:, 16 * c:16 * c + 16],
                                    rhs=plist[c][:, sl],
                                    start=(c == 0), stop=(c == NCHUNK - 1))
                                nc.tensor.matmul(
                                    ps2[:, sl],
                                    lhsT=mask[:, 16 * c:16 * c + 16],
                                    rhs=sq_list[(name, c)][:, sl],
                                    start=(c == 0), stop=(c == NCHUNK - 1))
                        s1 = statd.tile([16, N], F32, tag=f"s1{name}")
                        s2 = statd.tile([16, N], F32, tag=f"s2{name}")
                        nc.scalar.copy(s1, ps1)
                        nc.vector.tensor_copy(s2, ps2)
                        stats_sb[name] = (s1, s2)

            # ---- Stage D: stats math ----------------------------------
            s1q, s2q = stats_sb["q"]
            s1k, s2k = stats_sb["k"]
            invn_ap = invn_s[:, 0:1]

            # k-side: nssy = s1y^2/n - s2y = -ssy; beta = exp(-.5*ln(ssy))
            nc.vector.tensor_scalar(out=ns1y, in0=s1k, scalar1=-1.0,
                                    scalar2=None, op0=OP.mult)
            tk = statd.tile([16, N], F32, tag="tk")
            nc.vector.tensor_mul(tk, s1k, s1k)
            nssy = statd.tile([16, N], F32, tag="nssy")
            nc.vector.scalar_tensor_tensor(out=nssy, in0=tk, scalar=invn_ap,
                                           in1=s2k, op0=OP.mult,
                                           op1=OP.subtract)
            lssy = statd.tile([16, N], F32, tag="lssy")
            nc.scalar.activation(out=lssy, in_=nssy, func=AF.Ln,
                                 bias=0.0, scale=-1.0)
            beta16 = statd.tile([16, N], F32R, tag="beta16")
            nc.scalar.activation(out=beta16, in_=lssy, func=AF.Exp,
                                 bias=0.0, scale=-0.5)
            nc.sync.dma_start(out=betad[:, :], in_=beta16)

            # q-side: mx = s1x/n; a = exp(-.5*ln(ssx)); A = a^2
            nc.vector.tensor_scalar(out=mx, in0=s1q, scalar1=invn_ap,
                                    scalar2=None, op0=OP.mult)
            tq = statd.tile([16, N], F32, tag="tq")
            nc.vector.tensor_mul(tq, s1q, s1q)
            nssx = statd.tile([16, N], F32, tag="nssx")
            nc.vector.scalar_tensor_tensor(out=nssx, in0=tq, scalar=invn_ap,
                                           in1=s2q, op0=OP.mult,
                                           op1=OP.subtract)
            lssx = statd.tile([16, N], F32, tag="lssx")
            nc.scalar.activation(out=lssx, in_=nssx, func=AF.Ln,
                                 bias=0.0, scale=-1.0)
            a16 = statd.tile([16, N], F32, tag="a16")
            nc.scalar.activation(out=a16, in_=lssx, func=AF.Exp,
                                 bias=0.0, scale=-0.5)

            # transpose the scale table to [128, 8*16]: col ic*16 + r
            with tc.tile_pool(name="tps", bufs=1, space="PSUM") as tps:
                pst = tps.tile([128, 128], F32, tag="pst_a")
                for ic in range(8):
                    nc.tensor.transpose(pst[:, ic * 16:(ic + 1) * 16],
                                        in_=a16[:, ic * 128:(ic + 1) * 128],
                                        identity=id_s[0:16, 0:16])
                nc.scalar.copy(aT, pst)

        # m1 augmentation: overwrite q_projT row rb+63 (unused d=63) with mx1
        for h in range(H):
            ch, rb = h // 2, (h % 2) * 64
            nc.sync.dma_start(out=qproj[ch][rb + 63:rb + 64, :],
                              in_=mx[2 * h:2 * h + 1, :])

        # ---- Stage E: per-head Grams + elementwise (software-pipelined)
        # Flat iteration t = 8*h + ic; pair p = t//2, quad qd = t//4.
        #   step t+0: PE Gram matmuls -> psg1/psg2
        #   step t+1: evac1 ACT Square -> sgA half; evac2 DVE STT -> sgB half
        #   step t+2 (pair done): c1 = sgA - C; u = (sgB - C)*c1 -> uq half
        #   step t+4 (quad done): ACT Ln over [128, 4096]
        #   step t+5: DVE -0.5 (4x) -> o-quad
        #   step t+6: quad DMA store
        T = H * 8
        with tc.tile_pool(name="head", bufs=2) as headp, \
             tc.tile_pool(name="sga", bufs=2) as sgap, \
             tc.tile_pool(name="sgb", bufs=2) as sgbp, \
             tc.tile_pool(name="c1p", bufs=2) as c1p, \
             tc.tile_pool(name="uqp", bufs=2) as uqp, \
             tc.tile_pool(name="ltp", bufs=2) as ltp, \
             tc.tile_pool(name="oop", bufs=2) as oop, \
             tc.tile_pool(name="gps", bufs=2, space="PSUM") as gps:

            heads = {}

            def prep_head(h):
                ch, rb = h // 2, (h % 2) * 64
                yo1, yo2 = rb, 64 - rb
                r1, r2 = 2 * h, 2 * h + 1
                # Y raw: m1 block rows yo1..yo1+63 (k d=1..63 + aug),
                #        m2 block rows yo2..yo2+62 (k d=2..63 + aug)
                yraw = headp.tile([128, N], F32R, name="yraw", tag="yraw")
                nc.sync.dma_start(out=yraw[yo1:yo1 + 63, :],
                                  in_=kproj[ch][rb + 1:rb + 64, :])
                nc.sync.dma_start(out=yraw[yo1 + 63:yo1 + 64, :],
                                  in_=ns1y[r1:r1 + 1, :])
                nc.sync.dma_start(out=yraw[yo2:yo2 + 62, :],
                                  in_=kproj[ch][rb + 2:rb + 64, :])
                nc.sync.dma_start(out=yraw[yo2 + 62:yo2 + 63, :],
                                  in_=ns1y[r2:r2 + 1, :])
                hole = yo2 + 63  # the single uncovered row
                nc.sync.dma_start(out=yraw[hole:hole + 1, :],
                                  in_=ns1y[r1:r1 + 1, :])

                bb = headp.tile([128, N], F32R, name="bb", tag="bb")
                nc.gpsimd.dma_start(
                    out=bb[yo1:yo1 + 64, :],
                    in_=betad[r1:r1 + 1, :].to_broadcast((64, N)))
                nc.gpsimd.dma_start(
                    out=bb[yo2:yo2 + 64, :],
                    in_=betad[r2:r2 + 1, :].to_broadcast((64, N)))

                yt = headp.tile([128, N], F32R, name="yt", tag="yt")
                if h % 2 == 0:
                    nc.gpsimd.tensor_mul(yt, yraw, bb)
                else:
                    nc.vector.tensor_mul(yt, yraw, bb)

                # X2: m2 lhsT block at rows yo2..yo2+62 (q d=0..61 + mx2)
                x2 = headp.tile([128, N], F32R, name="x2", tag="x2")
                nc.sync.dma_start(out=x2[yo2:yo2 + 62, :],
                                  in_=qproj[ch][rb:rb + 62, :])
                nc.sync.dma_start(out=x2[yo2 + 62:yo2 + 63, :],
                                  in_=mx[r2:r2 + 1, :])
                heads[h] = (yt, x2)

            st = {}
            pairs = {}
            quads = {}

            def emit_pe(t):
                h, ic = divmod(t, 8)
                ch, rb = h // 2, (h % 2) * 64
                yo1, yo2 = rb, 64 - rb
                yt, x2 = heads[h]
                isl = slice(ic * 128, (ic + 1) * 128)
                psg1 = gps.tile([128, N], F32, name="psg1", tag="psg1")
                psg2 = gps.tile([128, N], F32, name="psg2", tag="psg2")
                for jh in range(2):
                    jsl = slice(jh * 512, (jh + 1) * 512)
                    nc.tensor.matmul(psg1[:, jsl],
                                     lhsT=qproj[ch][rb:rb + 64, isl],
                                     rhs=yt[yo1:yo1 + 64, jsl],
                                     start=True, stop=True)
                    nc.tensor.matmul(psg2[:, jsl],
                                     lhsT=x2[yo2:yo2 + 63, isl],
                                     rhs=yt[yo2:yo2 + 63, jsl],
                                     start=True, stop=True)
                st[t] = (psg1, psg2)

            def emit_evac(t):
                h, ic = divmod(t, 8)
                r1, r2 = 2 * h, 2 * h + 1
                p, half = divmod(t, 2)
                psg1, psg2 = st.pop(t)
                if half == 0:
                    pairs[p] = {
                        "sgA": sgap.tile([128, 2 * N], F16, name="sgA",
                                         tag="sgA"),
                        "sgB": sgbp.tile([128, 2 * N], F16, name="sgB",
                                         tag="sgB"),
                        "rB": sgbp.tile([128, 2 * N], F16, name="rB",
                                        tag="rB"),
                    }
                pw = pairs[p]
                sl = slice(half * N, (half + 1) * N)
                a1 = aT[:, ic * 16 + r1:ic * 16 + r1 + 1]
                # v1 = (a1*G1)^2 on ACT
                nc.scalar.activation(out=pw["sgA"][:, sl], in_=psg1,
                                     func=AF.Square, bias=0.0, scale=a1)
                a2 = aT[:, ic * 16 + r2:ic * 16 + r2 + 1]
                if (t % 8) in _ACT_EVAC2_SLOTS:
                    # balance: occasionally v2 = (a2*G2)^2 on ACT
                    nc.scalar.activation(out=pw["sgB"][:, sl], in_=psg2,
                                         func=AF.Square, bias=0.0, scale=a2)
                else:
                    # DVE path: r2 = a2*G2 (PSUM->f16), then square in f16
                    # (only one PSUM operand allowed per instruction)
                    rB = pw["rB"]
                    nc.vector.tensor_scalar(out=rB[:, sl], in0=psg2,
                                            scalar1=a2, scalar2=None,
                                            op0=OP.mult)
                    if (t % 8) in _POOL_SQ_SLOTS:
                        nc.gpsimd.tensor_mul(pw["sgB"][:, sl], rB[:, sl],
                                             rB[:, sl])
                    else:
                        nc.vector.tensor_mul(pw["sgB"][:, sl], rB[:, sl],
                                             rB[:, sl])

            def emit_pair(p):
                # c1 = v1 (-) C; u = (v2 (-) C)*c1. Both factors use the
                # same subtract convention, so u = +(v1-C)(v2-C) > 0
                # regardless of the ALU's operand order.
                pw = pairs.pop(p)
                qd, qhalf = divmod(p, 2)
                if qhalf == 0:
                    quads[qd] = {
                        "uq": uqp.tile([128, 4 * N], F16, name="uq",
                                       tag="uq"),
                    }
                uq = quads[qd]["uq"]
                c1 = c1p.tile([128, 2 * N], F16, name="c1", tag="c1")
                nc.vector.tensor_scalar(out=c1, in0=pw["sgA"], scalar1=C,
                                        scalar2=None, op0=OP.subtract)
                osl = slice(2 * qhalf * N, (2 * qhalf + 2) * N)
                nc.vector.scalar_tensor_tensor(
                    out=uq[:, osl], in0=pw["sgB"], scalar=C,
                    in1=c1, op0=OP.subtract, op1=OP.mult)

            def emit_ln(qd):
                qw = quads[qd]
                lt = ltp.tile([128, 4 * N], F16, name="lt", tag="lt")
                nc.scalar.activation(out=lt, in_=qw["uq"], func=AF.Ln,
                                     bias=0.0, scale=1.0)
                qw["lt"] = lt

            def emit_scale(qd):
                qw = quads[qd]
                ow = oop.tile([128, 4 * N], F16, name="ow", tag="ow")
                nc.vector.tensor_scalar(out=ow, in0=qw["lt"], scalar1=-0.5,
                                        scalar2=None, op0=OP.mult)
                qw["ow"] = ow

            def emit_store(qd):
                qw = quads.pop(qd)
                t0 = 4 * qd
                h, ic0 = divmod(t0, 8)
                dst = out[h, ic0 * 128:(ic0 + 4) * 128, :].rearrange(
                    "(t p) j -> p t j", t=4)
                src = qw["ow"][:, :].rearrange("p (t j) -> p t j", t=4)
                nc.sync.dma_start(out=dst, in_=src)

            prep_head(0)
            prep_head(1)
            for step in range(T + 7):
                if step < T:
                    emit_pe(step)
                    h, ic = divmod(step, 8)
                    if ic == 7 and h + 2 < H:
                        prep_head(h + 2)
                if 0 <= step - 1 < T:
                    emit_evac(step - 1)
                s = step - 2
                if 0 <= s < T and s % 2 == 1:
                    emit_pair((s - 1) // 2)
                s = step - 3
                if 0 <= s < T and s % 4 == 3:
                    emit_ln((s - 3) // 4)
                s = step - 4
                if 0 <= s < T and s % 4 == 3:
                    emit_scale((s - 3) // 4)
                s = step - 5
                if 0 <= s < T and s % 4 == 3:
                    emit_store((s - 3) // 4)


def _build_nc(repeat=1):
    nc = bacc.Bacc("TRN2", target_bir_lowering=False)

    qT = nc.dram_tensor("qT", [DF, N], F32, kind="ExternalInput")
    kT = nc.dram_tensor("kT", [DF, N], F32, kind="ExternalInput")
    wqT = nc.dram_tensor("wqT", [DF, 512], F32, kind="ExternalInput")
    wkT = nc.dram_tensor("wkT", [DF, 512], F32, kind="ExternalInput")
    bq = nc.dram_tensor("bq", [128, 4], F32, kind="ExternalInput")
    bk = nc.dram_tensor("bk", [128, 4], F32, kind="ExternalInput")
    xmask = nc.dram_tensor("xmask", [128, 64], F32, kind="ExternalInput")
    ymask = nc.dram_tensor("ymask", [128, 64], F32, kind="ExternalInput")
    invn = nc.dram_tensor("invn", [16, 1], F32, kind="ExternalInput")
    ident = nc.dram_tensor("ident", [128, 128], F32, kind="ExternalInput")
    out = nc.dram_tensor("out", [H, N, N], F16, kind="ExternalOutput")
    # DRAM bounce buffer for beta: SBUF sources cannot use partition-step-0
    # (broadcast) APs, DRAM sources can.
    betad = nc.dram_tensor("betad", [16, N], F32R, kind="Internal")

    t = (qT, kT, wqT, wkT, bq, bk, xmask, ymask, invn, ident, out, betad)
    with tile.TileContext(nc) as tc:
        for _rep in range(repeat):
            _emit_body(nc, tc, t)
    nc.compile()
    return nc


_NC = None


def _get_nc():
    global _NC
    if _NC is None:
        _NC = _build_nc()
    return _NC


def _host_inputs(queries, keys, Wq_w, Wq_b, Wk_w, Wk_b):
    qT = np.ascontiguousarray(queries.transpose(0, 2, 1), dtype=np.float32)
    kT = np.ascontiguousarray(keys.transpose(0, 2, 1), dtype=np.float32)
    wqT = np.ascontiguousarray(Wq_w.T, dtype=np.float32)
    wkT = np.ascontiguousarray(Wk_w.T, dtype=np.float32)
    bq = np.ascontiguousarray(Wq_b.reshape(4, 128).T, dtype=np.float32)
    bk = np.ascontiguousarray(Wk_b.reshape(4, 128).T, dtype=np.float32)

    xmask = np.zeros((128, 64), dtype=np.float32)
    ymask = np.zeros((128, 64), dtype=np.float32)
    for c in range(4):
        for hp in range(2):
            for m in (1, 2):
                j = 4 * c + 2 * hp + (m - 1)      # output partition row r
                col = 16 * c + j                   # column within this chunk's mask
                rows = np.arange(hp * 64, hp * 64 + 64 - m)
                xmask[rows, col] = 1.0
                yrows = np.arange(hp * 64 + m, hp * 64 + 64)
                ymask[yrows, col] = 1.0

    invn = np.array([[1.0 / (64 - ((r % 2) + 1))] for r in range(16)],
                    dtype=np.float32)
    ident = np.eye(128, dtype=np.float32)

    shared = dict(wqT=wqT, wkT=wkT, bq=bq, bk=bk, xmask=xmask, ymask=ymask,
                  invn=invn, ident=ident)
    in_maps = []
    for b in range(B):
        m = dict(shared)
        m["qT"] = np.ascontiguousarray(qT[b])
        m["kT"] = np.ascontiguousarray(kT[b])
        in_maps.append(m)
    return in_maps


def kernel(queries, keys, Wq_w, Wq_b, Wk_w, Wk_b):
    nc = _get_nc()
    in_maps = _host_inputs(np.asarray(queries), np.asarray(keys),
                           np.asarray(Wq_w), np.asarray(Wq_b),
                           np.asarray(Wk_w), np.asarray(Wk_b))
    res = run_bass_kernel_spmd(nc, in_maps, core_ids=list(range(B)))
    out = np.stack([res.results[b]["out"].astype(np.float32) for b in range(B)],
                   axis=0)
    return out
